# revision 14
# baseline (speedup 1.0000x reference)
"""ChaosNet (ChaosFEX + linear head) Trainium2 kernel.

Math restructure: every per-element feature depends only on k*(x) = first
trajectory index k with |traj[k] - x| < eps.  k*(x) is piecewise-constant in x
(first-claim intervals of the shared trajectory), so the model output

    out[n, c] = b_c + sum_f Phi_{c,f}(k*(x[n,f]))

is, per (c, f), a piecewise-constant function of x with M segments.  With
region left-edges L_0 <= ... <= L_{M-1} and per-segment table values Phi[m],
a telescoped form needs only rank indicators:

    Phi(x) = sum_m [x >= L_m] * dPhi[m]          (dPhi = successive deltas)

Device pipeline (per core, 256 rows of x, E = 8192 elements, mp regions in
the partition dim):
  - x is broadcast across the mp partitions in 512-column chunks, two ways:
      * PE: one ones-matmul per chunk over an exact 3-way bf16 split of x
        (x = hi+mid+lo exactly; the [3,mp] all-ones stationary reproduces x
        bit-exactly in PSUM at 1 PE cycle/column), or
      * gpsimd partition_broadcast from an f-major fp32 row (slower per
        column but runs on an otherwise idle engine).
  - compares u[m, j] = [x_j >= L_m] run on two engines:
      * DVE tensor_scalar is_ge -> fp16 {0,1}  (exact), or
      * Activation Sign(x - L) -> fp16 {-1,0,+1}; the (s+1)/2 re-encoding is
        folded into 0.5-scaled weight tables plus a per-channel constant,
        and the (measure-zero) x == L exact hits are patched on the host.
  - PE accumulates out[c, n] += sum_m u[m, f*256+n] * W'[m, 2f+c] over all 32
    f-blocks into one [4, 256] PSUM tile (fp16 hi/lo weight pairs).
  - DVE copies PSUM -> SBUF, one DMA out.

The host does only the inherently sequential scalar work: the 10000-step
trajectory, its prefix sums, and the exact-fp32 region partition (binary
search on fp32 bit patterns, so region edges reproduce the reference's
fp32 comparison semantics exactly).
"""

import os
import sys
from contextlib import ExitStack

import ml_dtypes
import numpy as np

sys.path.insert(0, "/opt/trn_rl_repo")

import concourse.bass as bass  # noqa: E402
import concourse.tile as tile  # noqa: E402
from concourse import bacc, mybir  # noqa: E402
from concourse.bass_utils import run_bass_kernel_spmd  # noqa: E402

T = 10000
N = 2048
F = 32
NCORES = 8
N_LOC = N // NCORES            # 256 rows per core
E = N_LOC * F                  # 8192 elements per core (f-major columns)
CHUNK = 1024                   # columns per pipeline chunk (= 4 f-blocks)
NCHUNK = E // CHUNK            # 8
FPC = CHUNK // N_LOC           # f-blocks per chunk (4)

np.seterr(all="ignore")

LAST_RESULTS = None            # BassKernelResults of the most recent run
LAST_NC = None                 # compiled Bass program of the most recent run


# ----------------------------------------------------------------------------
# Host-side preprocessing
# ----------------------------------------------------------------------------

def _build_traj(ic, thr):
    """fp32 skew-tent trajectory, bit-identical to the jax scan."""
    traj = np.empty(T, np.float32)
    z = np.float32(ic)
    thr = np.float32(thr)
    one = np.float32(1.0)
    omt = np.float32(one - thr)
    for k in range(T):
        traj[k] = z
        z = np.float32(z / thr) if z < thr else np.float32((one - z) / omt)
    return traj


def _sortable(i):
    """int32 bit pattern -> order-isomorphic int32 key (handles negatives)."""
    return np.where(i >= 0, i, i ^ np.int32(0x7FFFFFFF))


def _unsortable(k):
    return np.where(k >= 0, k, k ^ np.int32(0x7FFFFFFF))


def _match_intervals(traj, eps, xmin, xmax):
    """Exact fp32 interval [lo_k, hi_k] of {x in [xmin,xmax] :
    |fl32(traj_k - x)| < eps}; valid[k]=False if empty."""
    eps = np.float32(eps)
    xmin = np.float32(xmin)
    xmax = np.float32(xmax)

    def cond(xs):
        return np.abs(traj - xs.astype(np.float32)) < eps

    anchor = np.clip(traj, xmin, xmax)
    valid = cond(anchor)

    I = lambda f: _sortable(f.view(np.int32))             # noqa: E731
    Fv = lambda k: _unsortable(k).view(np.float32)        # noqa: E731

    def bisect(lo_i, hi_i, need, want_smallest_true):
        # invariant: cond(Fv(hi_i)) True/False per direction; int keys.
        for _ in range(40):
            gap = np.where(need, hi_i - lo_i, 0)
            if (gap <= 1).all():
                break
            mid = ((lo_i.astype(np.int64) + hi_i) // 2).astype(np.int32)
            cm = cond(Fv(mid))
            if want_smallest_true:
                hi_i = np.where(need & cm, mid, hi_i)
                lo_i = np.where(need & ~cm, mid, lo_i)
            else:
                lo_i = np.where(need & cm, mid, lo_i)
                hi_i = np.where(need & ~cm, mid, hi_i)
        return lo_i, hi_i

    # left edge: smallest x in [xmin, anchor] with cond True
    at_min = cond(np.full(T, xmin, np.float32))
    lo_edge = np.where(at_min, xmin, np.float32(np.nan))
    need = valid & np.isnan(lo_edge)
    lo_i = np.broadcast_to(I(xmin.reshape(1)), (T,)).copy()
    hi_i = I(anchor.copy())
    lo_i, hi_i = bisect(lo_i, hi_i, need, True)
    lo_edge = np.where(np.isnan(lo_edge), Fv(hi_i), lo_edge)

    # right edge: largest x in [anchor, xmax] with cond True
    at_max = cond(np.full(T, xmax, np.float32))
    hi_edge = np.where(at_max, xmax, np.float32(np.nan))
    need = valid & np.isnan(hi_edge)
    lo_i = I(anchor.copy())
    hi_i = np.broadcast_to(I(xmax.reshape(1)), (T,)).copy()
    lo_i, hi_i = bisect(lo_i, hi_i, need, False)
    hi_edge = np.where(np.isnan(hi_edge), Fv(lo_i), hi_edge)

    # exactness checks (cheap, vectorized)
    v = valid
    assert cond(np.where(v, lo_edge, anchor)).all()
    assert cond(np.where(v, hi_edge, anchor)).all()
    below = np.nextafter(lo_edge, np.float32(-np.inf))
    above = np.nextafter(hi_edge, np.float32(np.inf))
    assert not (v & (below >= xmin) & cond(below)).any()
    assert not (v & (above <= xmax) & cond(above)).any()
    return lo_edge, hi_edge, valid


def _build_regions(traj, eps, xmin, xmax):
    """First-claim partition of [xmin, xmax] into regions of constant k*.
    Returns sorted left edges L (fp32) and per-region kstar (== T: never)."""
    xl, xr, valid = _match_intervals(traj, eps, xmin, xmax)
    down = lambda a: np.nextafter(a, np.float32(-np.inf))  # noqa: E731
    up = lambda a: np.nextafter(a, np.float32(np.inf))     # noqa: E731
    uncovered = [(np.float32(xmin), np.float32(xmax))]
    regions = []
    for k in range(T):
        if not uncovered:
            break
        if not valid[k]:
            continue
        lo_k, hi_k = xl[k], xr[k]
        new_unc = []
        for (a, b) in uncovered:
            if lo_k > b or hi_k < a:
                new_unc.append((a, b))
                continue
            ra, rb = max(lo_k, a), min(hi_k, b)
            regions.append((ra, k))
            if a < ra:
                new_unc.append((a, down(ra)))
            if rb < b:
                new_unc.append((up(rb), b))
        uncovered = new_unc
    for (a, b) in uncovered:
        regions.append((a, T))
    regions.sort(key=lambda r: r[0])
    L = np.array([r[0] for r in regions], np.float32)
    ks = np.array([r[1] for r in regions], np.int64)
    return L, ks


def _region_features(traj, thr, ks):
    """Per-region (tt, energy, p, ent) with the reference's fp32 accumulation
    semantics (sequential fp32 cumsum == per-step fp32 adds)."""
    thr = np.float32(thr)
    t2 = traj * traj                                  # fp32 squares
    Ecum = np.cumsum(t2, dtype=np.float32)            # sequential fp32 adds
    gt = (traj > thr).astype(np.float32)
    Ccum = np.cumsum(gt, dtype=np.float32)            # exact small ints
    fired = ks < T
    j = np.where(fired, ks, T - 1)
    tt = np.where(fired, ks + 1, T).astype(np.float32)
    en = Ecum[j].astype(np.float32)
    cnt = Ccum[j].astype(np.float32)
    p = (cnt / tt).astype(np.float32)

    def xlog2x(v):
        safe = np.where(v > 0, v, np.float32(1.0)).astype(np.float32)
        return np.where(v > 0, v * np.log2(safe, dtype=np.float32),
                        np.float32(0.0)).astype(np.float32)

    ent = -(xlog2x(p) + xlog2x((np.float32(1.0) - p).astype(np.float32)))
    return tt, en, p, ent.astype(np.float32)


def _split_bf16_3(x32):
    """Exact 3-way bf16 split: x == hi + mid + lo (verified)."""
    bf = ml_dtypes.bfloat16
    hi = x32.astype(bf)
    r1 = (x32 - hi.astype(np.float32)).astype(np.float32)
    mid = r1.astype(bf)
    r2 = (r1 - mid.astype(np.float32)).astype(np.float32)
    lo = r2.astype(bf)
    recon = ((hi.astype(np.float32) + mid.astype(np.float32))
             + lo.astype(np.float32)).astype(np.float32)
    assert np.array_equal(recon, x32), "3-way bf16 split is not exact"
    recon2 = (hi.astype(np.float32)
              + (mid.astype(np.float32) + lo.astype(np.float32)))
    assert np.array_equal(recon2.astype(np.float32), x32), \
        "3-way bf16 split order-sensitive"
    return hi, mid, lo


# Per-chunk routing.  BCAST[k] in {"pe", "gps"}; CMP[k] in {"dve", "act"}.
# "act" chunks use the Sign encoding (0.5-scaled tables + constant).
def _routes():
    gps = os.environ.get("GPS_CHUNKS", "3,6")
    act = os.environ.get("ACT_CHUNKS", "0,2,5,7")
    gps = set(int(s) for s in gps.split(",") if s != "")
    act = set(int(s) for s in act.split(",") if s != "")
    bcast = ["gps" if k in gps else "pe" for k in range(NCHUNK)]
    cmp_ = ["act" if k in act else "dve" for k in range(NCHUNK)]
    return bcast, cmp_


def _build_tables(x, ic, thr, eps, W, b):
    """Builds all device-side tables plus host-side output corrections."""
    traj = _build_traj(ic, thr)
    L, ks = _build_regions(traj, eps, float(x.min()), float(x.max()))
    tt, en, p, ent = _region_features(traj, thr, ks)
    M = L.shape[0]
    assert M <= 128, f"region count {M} exceeds one partition block"

    # Phi[m, 2f+c] = W[c,4f]*tt + W[c,4f+1]*en + W[c,4f+2]*p + W[c,4f+3]*ent
    W64 = W.astype(np.float64).reshape(2, F, 4)
    feats64 = np.stack([tt, en, p, ent], -1).astype(np.float64)   # [M, 4]
    phi = np.einsum("mj,cfj->mcf", feats64, W64)                  # [M, 2, F]
    phi = phi.transpose(0, 2, 1).reshape(M, 2 * F)                # [M, 64]

    # compensated fp32 deltas: partial fp32 sums track the fp64 table
    dphi = np.empty((M, 2 * F), np.float32)
    running = np.zeros(2 * F, np.float64)
    for m in range(M):
        d = (phi[m] - running).astype(np.float32)
        dphi[m] = d
        running += d.astype(np.float64)

    # pad M to a multiple of 8 partitions; L pad = +inf (never <= x)
    mp = max(16, ((M + 7) // 8) * 8)
    L_pad = np.full(mp, np.float32(np.inf), np.float32)
    L_pad[:M] = L
    dphi_pad = np.zeros((mp, 2 * F), np.float32)
    dphi_pad[:M] = dphi

    def pack_hilo(d32):
        """[mp, 2F] fp32 -> [mp, 4F] fp16: per f (hi_c0, hi_c1, lo_c0, lo_c1)."""
        hi16 = d32.astype(np.float16)
        lo16 = (d32.astype(np.float64) - hi16.astype(np.float64)) \
            .astype(np.float16)
        out = np.empty((mp, 4 * F), np.float16)
        for f in range(F):
            out[:, 4 * f:4 * f + 2] = hi16[:, 2 * f:2 * f + 2]
            out[:, 4 * f + 2:4 * f + 4] = lo16[:, 2 * f:2 * f + 2]
        return out

    whi = pack_hilo(dphi_pad)                        # is_ge chunks
    whs = pack_hilo(0.5 * dphi_pad)                  # Sign chunks (0.5-scaled)

    # consts [mp, 130] fp32: col0 = L, col1 = -L, cols 2:66 = whi (f16 pairs
    # viewed as f32 words), cols 66:130 = whs
    consts = np.zeros((mp, 130), np.float32)
    consts[:, 0] = L_pad
    consts[:, 1] = -L_pad
    consts[:, 2:66] = whi.view(np.float32)
    consts[:, 66:130] = whs.view(np.float32)

    bcast_r, cmp_r = _routes()
    # Sign-path constant per channel: K_c = sum over sign-chunk features f of
    # sum_m [(0.5 d)_hi + (0.5 d)_lo]  (from the actual device fp16 tables)
    K = np.zeros(2, np.float64)
    sign_f = [f for f in range(F) if cmp_r[f // FPC] == "act"]
    for f in sign_f:
        for c in range(2):
            K[c] += (whs[:, 4 * f + c].astype(np.float64).sum()
                     + whs[:, 4 * f + 2 + c].astype(np.float64).sum())

    # exact x == L hits on Sign-path features lose 0.5*dphi (sign(0) = 0)
    corrections = []                                 # (n, f, m) triples
    hit_rows, hit_fs = np.nonzero(np.isin(x, L[:M]))
    for n, f in zip(hit_rows, hit_fs):
        if cmp_r[f // FPC] != "act":
            continue
        m = int(np.nonzero(L[:M] == x[n, f])[0][0])
        corrections.append((int(n), int(f), m))

    return consts, whi, whs, mp, K, corrections, bcast_r, cmp_r


# ----------------------------------------------------------------------------
# Device kernel
# ----------------------------------------------------------------------------

def _build_device_program(mp, bcast_r, cmp_r):
    nc = bacc.Bacc("TRN2", target_bir_lowering=False, debug=False,
                   num_devices=NCORES)
    f32 = mybir.dt.float32
    f16 = mybir.dt.float16
    bf16 = mybir.dt.bfloat16
    is_ge = mybir.AluOpType.is_ge
    Sign = mybir.ActivationFunctionType.Sign

    xs_d = nc.dram_tensor("xs", [3, E + mp], bf16, kind="ExternalInput").ap()
    ct_d = nc.dram_tensor("ct", [mp, 130], f32, kind="ExternalInput").ap()
    need_xf = any(r == "gps" for r in bcast_r)
    if need_xf:
        xf_d = nc.dram_tensor("xf", [1, E], f32, kind="ExternalInput").ap()
    out_d = nc.dram_tensor("out", [4, N_LOC], f32, kind="ExternalOutput").ap()

    with tile.TileContext(nc) as tc, ExitStack() as ctx:
        consts = ctx.enter_context(tc.tile_pool(name="consts", bufs=1))
        warmp = ctx.enter_context(tc.tile_pool(name="warm", bufs=1))
        gpb = ctx.enter_context(tc.tile_pool(name="gpb", bufs=2))
        u16p = ctx.enter_context(tc.tile_pool(name="u16", bufs=4))
        outp = ctx.enter_context(tc.tile_pool(name="outp", bufs=1))
        psum = ctx.enter_context(tc.tile_pool(name="psum", bufs=2,
                                              space="PSUM"))
        psacc = ctx.enter_context(tc.tile_pool(name="psacc", bufs=1,
                                               space="PSUM"))
        pswarm = ctx.enter_context(tc.tile_pool(name="pswarm", bufs=1,
                                                space="PSUM"))

        # ---- input DMAs (dispatch order = HWDGE order) -----------------
        xs = consts.tile([3, E + mp], bf16, tag="xs")
        nc.sync.dma_start(xs[:, :], xs_d)            # SP queue, fastest decode
        ct = consts.tile([mp, 130], f32, tag="ct")
        nc.scalar.dma_start(ct[:, :], ct_d)          # Activation queue
        if need_xf:
            xf = consts.tile([1, E], f32, tag="xf")
            nc.gpsimd.dma_start(xf[:, :], xf_d)      # SWDGE, Pool engine

        lpe = ct[:, 0:1]
        nlpe = ct[:, 1:2]
        whi = ct[:, 2:66].bitcast(f16)               # [mp, 128]
        whs = ct[:, 66:130].bitcast(f16)
        ones3 = xs[:, E:E + mp]                      # [3, mp] all-ones bf16

        # ---- warmup: act table load + PE pstate ramp during the DMA head
        wb = warmp.tile([3, 2], bf16, tag="wb")
        nc.gpsimd.memset(wb[:, :], 0.0)
        wf = warmp.tile([1, 2], f32, tag="wf")
        nc.gpsimd.memset(wf[:, :], 0.0)
        wo = warmp.tile([1, 2], f16, tag="wo")
        nc.scalar.activation(wo[:, :], wf[:, :], Sign, bias=0.0, scale=1.0)
        pw = pswarm.tile([2, 2], f32, tag="pw")
        nc.tensor.matmul(pw[:, :], wb[:, 0:2], wb[:, 0:2],
                         start=True, stop=True)

        # ---- pipeline ---------------------------------------------------
        xb_ps = {}                                   # chunk -> PSUM tile
        xb_sb = {}                                   # chunk -> SBUF tile
        u16 = {}
        acc = psacc.tile([4, N_LOC], f32, tag="acc")

        def emit_bcast(k):
            sl = slice(k * CHUNK, (k + 1) * CHUNK)
            if bcast_r[k] == "pe":
                t = psum.tile([mp, CHUNK], f32, tag="xb", name=f"xb{k}")
                # one matmul per 512-col half: PSUM-bank limit
                for h in range(CHUNK // 512):
                    hs = slice(k * CHUNK + h * 512, k * CHUNK + (h + 1) * 512)
                    nc.tensor.matmul(t[:, h * 512:(h + 1) * 512],
                                     ones3[:, :], xs[:, hs],
                                     start=True, stop=True)
                xb_ps[k] = t
            else:
                t = gpb.tile([mp, CHUNK], f32, tag="xg", name=f"xg{k}")
                nc.gpsimd.partition_broadcast(t[:, :], xf[:, sl])
                xb_sb[k] = t

        def emit_cmp(k):
            src = xb_ps[k] if k in xb_ps else xb_sb[k]
            u = u16p.tile([mp, CHUNK], f16, tag="u", name=f"u{k}")
            if cmp_r[k] == "dve":
                nc.vector.tensor_scalar(u[:, :], src[:, :], lpe, None, is_ge)
            else:
                nc.scalar.activation(u[:, :], src[:, :], Sign,
                                     bias=nlpe, scale=1.0)
            u16[k] = u

        acc_started = [False]

        def emit_acc(k):
            tbl = whi if cmp_r[k] == "dve" else whs
            for half in range(FPC):
                f = FPC * k + half
                usl = u16[k][:, half * N_LOC:(half + 1) * N_LOC]
                last = (k == NCHUNK - 1 and half == FPC - 1)
                nc.tensor.matmul(acc[:, :], tbl[:, 4 * f:4 * f + 4], usl,
                                 start=not acc_started[0], stop=last)
                acc_started[0] = True

        pe_chunks = [k for k in range(NCHUNK) if bcast_r[k] == "pe"]
        gps_chunks = [k for k in range(NCHUNK) if bcast_r[k] == "gps"]
        # gpsimd broadcasts stream independently on the Pool engine
        for k in gps_chunks:
            emit_bcast(k)
        # PE: keep 2 broadcast chunks in flight ahead of the accumulates
        lookahead = 2
        for k in pe_chunks[:lookahead]:
            emit_bcast(k)
        nxt = lookahead
        for k in range(NCHUNK):
            emit_cmp(k)
            emit_acc(k)
            if bcast_r[k] == "pe" and nxt < len(pe_chunks):
                emit_bcast(pe_chunks[nxt])
                nxt += 1

        # ---- output -----------------------------------------------------
        outs = outp.tile([4, N_LOC], f32)
        nc.vector.tensor_scalar(outs[:, :], acc[:, :], 0.0, None,
                                mybir.AluOpType.add)
        nc.sync.dma_start(out_d, outs[:, :])

    nc.compile()
    return nc


# ----------------------------------------------------------------------------
# Entry point
# ----------------------------------------------------------------------------

def kernel(x, initial_cond, threshold, epsilon, W, b):
    global LAST_RESULTS, LAST_NC
    x = np.ascontiguousarray(np.asarray(x, np.float32))
    W = np.asarray(W, np.float32)
    b = np.asarray(b, np.float32)
    ic = float(np.asarray(initial_cond).reshape(-1)[0])
    thr = float(np.asarray(threshold).reshape(-1)[0])
    eps = float(np.asarray(epsilon).reshape(-1)[0])

    consts, whi, whs, mp, K, corrections, bcast_r, cmp_r = _build_tables(
        x, ic, thr, eps, W, b)
    need_xf = any(r == "gps" for r in bcast_r)

    nc = _build_device_program(mp, bcast_r, cmp_r)
    LAST_NC = nc

    bf = ml_dtypes.bfloat16
    in_maps = []
    for d in range(NCORES):
        xd = x[d * N_LOC:(d + 1) * N_LOC, :]         # [256, 32]
        xrow = np.ascontiguousarray(xd.T).reshape(E)  # f-major
        hi, mid, lo = _split_bf16_3(xrow)
        xsplit = np.ones((3, E + mp), bf)
        xsplit[0, :E] = hi
        xsplit[1, :E] = mid
        xsplit[2, :E] = lo
        im = {"xs": xsplit, "ct": consts}
        if need_xf:
            im["xf"] = xrow.reshape(1, E)
        in_maps.append(im)

    res = run_bass_kernel_spmd(nc, in_maps, core_ids=list(range(NCORES)))
    LAST_RESULTS = res

    out = np.empty((N, 2), np.float64)
    for d in range(NCORES):
        o4 = res.results[d]["out"].astype(np.float64)  # [4, 256] hi/lo rows
        out[d * N_LOC:(d + 1) * N_LOC, :] = (o4[:2] + o4[2:]).T
    out += b.astype(np.float64).reshape(1, 2) + K.reshape(1, 2)
    for (n, f, m) in corrections:
        for c in range(2):
            out[n, c] += (float(whs[m, 4 * f + c])
                          + float(whs[m, 4 * f + 2 + c]))
    return out.astype(np.float32)


# revision 20
# speedup vs baseline: 1.0648x; 1.0648x over previous
"""ChaosNet (ChaosFEX + linear head) Trainium2 kernel.

Math restructure: every per-element feature depends only on k*(x) = first
trajectory index k with |traj[k] - x| < eps.  k*(x) is piecewise-constant in x
(first-claim intervals of the shared trajectory), so the model output

    out[n, c] = b_c + sum_f Phi_{c,f}(k*(x[n,f]))

is, per (c, f), a piecewise-constant function of x with M segments.  With
region left-edges L_0 <= ... <= L_{M-1} and per-segment table values Phi[m],
a telescoped form needs only rank indicators:

    Phi(x) = sum_m [x >= L_m] * dPhi[m]          (dPhi = successive deltas)

Device pipeline (per core, 256 rows of x, E = 8192 elements, mp regions in
the partition dim):
  - x is broadcast across the mp partitions in 512-column chunks, two ways:
      * PE: one ones-matmul per chunk over an exact 3-way bf16 split of x
        (x = hi+mid+lo exactly; the [3,mp] all-ones stationary reproduces x
        bit-exactly in PSUM at 1 PE cycle/column), or
      * gpsimd partition_broadcast from an f-major fp32 row (slower per
        column but runs on an otherwise idle engine).
  - compares u[m, j] = [x_j >= L_m] run on two engines:
      * DVE tensor_scalar is_ge -> fp16 {0,1}  (exact), or
      * Activation Sign(x - L) -> fp16 {-1,0,+1}; the (s+1)/2 re-encoding is
        folded into 0.5-scaled weight tables plus a per-channel constant,
        and the (measure-zero) x == L exact hits are patched on the host.
  - PE accumulates out[c, n] += sum_m u[m, f*256+n] * W'[m, 2f+c] over all 32
    f-blocks into one [4, 256] PSUM tile (fp16 hi/lo weight pairs).
  - DVE copies PSUM -> SBUF, one DMA out.

The host does only the inherently sequential scalar work: the 10000-step
trajectory, its prefix sums, and the exact-fp32 region partition (binary
search on fp32 bit patterns, so region edges reproduce the reference's
fp32 comparison semantics exactly).
"""

import os
import sys
from contextlib import ExitStack

import ml_dtypes
import numpy as np

sys.path.insert(0, "/opt/trn_rl_repo")

import concourse.bass as bass  # noqa: E402
import concourse.tile as tile  # noqa: E402
from concourse import bacc, mybir  # noqa: E402
from concourse.bass_utils import run_bass_kernel_spmd  # noqa: E402

T = 10000
N = 2048
F = 32
NCORES = 8
N_LOC = N // NCORES            # 256 rows per core
E = N_LOC * F                  # 8192 elements per core (f-major columns)
CHUNK = 1024                   # columns per pipeline chunk (= 4 f-blocks)
NCHUNK = E // CHUNK            # 8
FPC = CHUNK // N_LOC           # f-blocks per chunk (4)

np.seterr(all="ignore")

LAST_RESULTS = None            # BassKernelResults of the most recent run
LAST_NC = None                 # compiled Bass program of the most recent run


# ----------------------------------------------------------------------------
# Host-side preprocessing
# ----------------------------------------------------------------------------

def _build_traj(ic, thr):
    """fp32 skew-tent trajectory, bit-identical to the jax scan."""
    traj = np.empty(T, np.float32)
    z = np.float32(ic)
    thr = np.float32(thr)
    one = np.float32(1.0)
    omt = np.float32(one - thr)
    for k in range(T):
        traj[k] = z
        z = np.float32(z / thr) if z < thr else np.float32((one - z) / omt)
    return traj


def _sortable(i):
    """int32 bit pattern -> order-isomorphic int32 key (handles negatives)."""
    return np.where(i >= 0, i, i ^ np.int32(0x7FFFFFFF))


def _unsortable(k):
    return np.where(k >= 0, k, k ^ np.int32(0x7FFFFFFF))


def _match_intervals(traj, eps, xmin, xmax):
    """Exact fp32 interval [lo_k, hi_k] of {x in [xmin,xmax] :
    |fl32(traj_k - x)| < eps}; valid[k]=False if empty."""
    eps = np.float32(eps)
    xmin = np.float32(xmin)
    xmax = np.float32(xmax)

    def cond(xs):
        return np.abs(traj - xs.astype(np.float32)) < eps

    anchor = np.clip(traj, xmin, xmax)
    valid = cond(anchor)

    I = lambda f: _sortable(f.view(np.int32))             # noqa: E731
    Fv = lambda k: _unsortable(k).view(np.float32)        # noqa: E731

    def bisect(lo_i, hi_i, need, want_smallest_true):
        # invariant: cond(Fv(hi_i)) True/False per direction; int keys.
        for _ in range(40):
            gap = np.where(need, hi_i - lo_i, 0)
            if (gap <= 1).all():
                break
            mid = ((lo_i.astype(np.int64) + hi_i) // 2).astype(np.int32)
            cm = cond(Fv(mid))
            if want_smallest_true:
                hi_i = np.where(need & cm, mid, hi_i)
                lo_i = np.where(need & ~cm, mid, lo_i)
            else:
                lo_i = np.where(need & cm, mid, lo_i)
                hi_i = np.where(need & ~cm, mid, hi_i)
        return lo_i, hi_i

    # left edge: smallest x in [xmin, anchor] with cond True
    at_min = cond(np.full(T, xmin, np.float32))
    lo_edge = np.where(at_min, xmin, np.float32(np.nan))
    need = valid & np.isnan(lo_edge)
    lo_i = np.broadcast_to(I(xmin.reshape(1)), (T,)).copy()
    hi_i = I(anchor.copy())
    lo_i, hi_i = bisect(lo_i, hi_i, need, True)
    lo_edge = np.where(np.isnan(lo_edge), Fv(hi_i), lo_edge)

    # right edge: largest x in [anchor, xmax] with cond True
    at_max = cond(np.full(T, xmax, np.float32))
    hi_edge = np.where(at_max, xmax, np.float32(np.nan))
    need = valid & np.isnan(hi_edge)
    lo_i = I(anchor.copy())
    hi_i = np.broadcast_to(I(xmax.reshape(1)), (T,)).copy()
    lo_i, hi_i = bisect(lo_i, hi_i, need, False)
    hi_edge = np.where(np.isnan(hi_edge), Fv(lo_i), hi_edge)

    # exactness checks (cheap, vectorized)
    v = valid
    assert cond(np.where(v, lo_edge, anchor)).all()
    assert cond(np.where(v, hi_edge, anchor)).all()
    below = np.nextafter(lo_edge, np.float32(-np.inf))
    above = np.nextafter(hi_edge, np.float32(np.inf))
    assert not (v & (below >= xmin) & cond(below)).any()
    assert not (v & (above <= xmax) & cond(above)).any()
    return lo_edge, hi_edge, valid


def _build_regions(traj, eps, xmin, xmax):
    """First-claim partition of [xmin, xmax] into regions of constant k*.
    Returns sorted left edges L (fp32) and per-region kstar (== T: never)."""
    xl, xr, valid = _match_intervals(traj, eps, xmin, xmax)
    down = lambda a: np.nextafter(a, np.float32(-np.inf))  # noqa: E731
    up = lambda a: np.nextafter(a, np.float32(np.inf))     # noqa: E731
    uncovered = [(np.float32(xmin), np.float32(xmax))]
    regions = []
    for k in range(T):
        if not uncovered:
            break
        if not valid[k]:
            continue
        lo_k, hi_k = xl[k], xr[k]
        new_unc = []
        for (a, b) in uncovered:
            if lo_k > b or hi_k < a:
                new_unc.append((a, b))
                continue
            ra, rb = max(lo_k, a), min(hi_k, b)
            regions.append((ra, k))
            if a < ra:
                new_unc.append((a, down(ra)))
            if rb < b:
                new_unc.append((up(rb), b))
        uncovered = new_unc
    for (a, b) in uncovered:
        regions.append((a, T))
    regions.sort(key=lambda r: r[0])
    L = np.array([r[0] for r in regions], np.float32)
    ks = np.array([r[1] for r in regions], np.int64)
    return L, ks


def _region_features(traj, thr, ks):
    """Per-region (tt, energy, p, ent) with the reference's fp32 accumulation
    semantics (sequential fp32 cumsum == per-step fp32 adds)."""
    thr = np.float32(thr)
    t2 = traj * traj                                  # fp32 squares
    Ecum = np.cumsum(t2, dtype=np.float32)            # sequential fp32 adds
    gt = (traj > thr).astype(np.float32)
    Ccum = np.cumsum(gt, dtype=np.float32)            # exact small ints
    fired = ks < T
    j = np.where(fired, ks, T - 1)
    tt = np.where(fired, ks + 1, T).astype(np.float32)
    en = Ecum[j].astype(np.float32)
    cnt = Ccum[j].astype(np.float32)
    p = (cnt / tt).astype(np.float32)

    def xlog2x(v):
        safe = np.where(v > 0, v, np.float32(1.0)).astype(np.float32)
        return np.where(v > 0, v * np.log2(safe, dtype=np.float32),
                        np.float32(0.0)).astype(np.float32)

    ent = -(xlog2x(p) + xlog2x((np.float32(1.0) - p).astype(np.float32)))
    return tt, en, p, ent.astype(np.float32)


def _split_bf16_3(x32):
    """Exact 3-way bf16 split: x == hi + mid + lo (verified)."""
    bf = ml_dtypes.bfloat16
    hi = x32.astype(bf)
    r1 = (x32 - hi.astype(np.float32)).astype(np.float32)
    mid = r1.astype(bf)
    r2 = (r1 - mid.astype(np.float32)).astype(np.float32)
    lo = r2.astype(bf)
    recon = ((hi.astype(np.float32) + mid.astype(np.float32))
             + lo.astype(np.float32)).astype(np.float32)
    assert np.array_equal(recon, x32), "3-way bf16 split is not exact"
    recon2 = (hi.astype(np.float32)
              + (mid.astype(np.float32) + lo.astype(np.float32)))
    assert np.array_equal(recon2.astype(np.float32), x32), \
        "3-way bf16 split order-sensitive"
    return hi, mid, lo


# Per-chunk routing.  BCAST[k] in {"pe", "gps"}; CMP[k] in {"dve", "act"}.
# "act" chunks use the Sign encoding (0.5-scaled tables + constant).
def _routes():
    gps = os.environ.get("GPS_CHUNKS", "5,6,7")
    act = os.environ.get("ACT_CHUNKS", "0,2,3,4")
    gps = set(int(s) for s in gps.split(",") if s != "")
    act = set(int(s) for s in act.split(",") if s != "")
    bcast = ["gps" if k in gps else "pe" for k in range(NCHUNK)]
    cmp_ = ["act" if k in act else "dve" for k in range(NCHUNK)]
    return bcast, cmp_


def _acc_order():
    """PE accumulation order: gate the tail on early-completing compares."""
    s = os.environ.get("ACC_ORDER", "0,5,1,2,6,3,7,4")
    order = [int(v) for v in s.split(",")]
    assert sorted(order) == list(range(NCHUNK))
    return order


def _build_tables(x, ic, thr, eps, W, b):
    """Builds all device-side tables plus host-side output corrections."""
    traj = _build_traj(ic, thr)
    L, ks = _build_regions(traj, eps, float(x.min()), float(x.max()))
    tt, en, p, ent = _region_features(traj, thr, ks)
    M = L.shape[0]
    assert M <= 128, f"region count {M} exceeds one partition block"

    # Phi[m, 2f+c] = W[c,4f]*tt + W[c,4f+1]*en + W[c,4f+2]*p + W[c,4f+3]*ent
    W64 = W.astype(np.float64).reshape(2, F, 4)
    feats64 = np.stack([tt, en, p, ent], -1).astype(np.float64)   # [M, 4]
    phi = np.einsum("mj,cfj->mcf", feats64, W64)                  # [M, 2, F]
    phi = phi.transpose(0, 2, 1).reshape(M, 2 * F)                # [M, 64]

    # compensated fp32 deltas: partial fp32 sums track the fp64 table
    dphi = np.empty((M, 2 * F), np.float32)
    running = np.zeros(2 * F, np.float64)
    for m in range(M):
        d = (phi[m] - running).astype(np.float32)
        dphi[m] = d
        running += d.astype(np.float64)

    # pad M to a multiple of 8 partitions; L pad = +inf (never <= x)
    mp = max(16, ((M + 7) // 8) * 8)
    L_pad = np.full(mp, np.float32(np.inf), np.float32)
    L_pad[:M] = L
    dphi_pad = np.zeros((mp, 2 * F), np.float32)
    dphi_pad[:M] = dphi

    def pack_hilo(d32):
        """[mp, 2F] fp32 -> [mp, 4F] fp16: per f (hi_c0, hi_c1, lo_c0, lo_c1)."""
        hi16 = d32.astype(np.float16)
        lo16 = (d32.astype(np.float64) - hi16.astype(np.float64)) \
            .astype(np.float16)
        out = np.empty((mp, 4 * F), np.float16)
        for f in range(F):
            out[:, 4 * f:4 * f + 2] = hi16[:, 2 * f:2 * f + 2]
            out[:, 4 * f + 2:4 * f + 4] = lo16[:, 2 * f:2 * f + 2]
        return out

    whi = pack_hilo(dphi_pad)                        # is_ge chunks
    whs = pack_hilo(0.5 * dphi_pad)                  # Sign chunks (0.5-scaled)

    # consts [mp, 130] fp32: col0 = L, col1 = -L, cols 2:66 = whi (f16 pairs
    # viewed as f32 words), cols 66:130 = whs
    consts = np.zeros((mp, 130), np.float32)
    consts[:, 0] = L_pad
    consts[:, 1] = -L_pad
    consts[:, 2:66] = whi.view(np.float32)
    consts[:, 66:130] = whs.view(np.float32)

    bcast_r, cmp_r = _routes()
    # Sign-path constant per channel: K_c = sum over sign-chunk features f of
    # sum_m [(0.5 d)_hi + (0.5 d)_lo]  (from the actual device fp16 tables)
    K = np.zeros(2, np.float64)
    sign_f = [f for f in range(F) if cmp_r[f // FPC] == "act"]
    for f in sign_f:
        for c in range(2):
            K[c] += (whs[:, 4 * f + c].astype(np.float64).sum()
                     + whs[:, 4 * f + 2 + c].astype(np.float64).sum())

    # exact x == L hits on Sign-path features lose 0.5*dphi (sign(0) = 0)
    corrections = []                                 # (n, f, m) triples
    hit_rows, hit_fs = np.nonzero(np.isin(x, L[:M]))
    for n, f in zip(hit_rows, hit_fs):
        if cmp_r[f // FPC] != "act":
            continue
        m = int(np.nonzero(L[:M] == x[n, f])[0][0])
        corrections.append((int(n), int(f), m))

    return consts, whi, whs, mp, K, corrections, bcast_r, cmp_r


# ----------------------------------------------------------------------------
# Device kernel
# ----------------------------------------------------------------------------

def _build_device_program(mp, bcast_r, cmp_r):
    nc = bacc.Bacc("TRN2", target_bir_lowering=False, debug=False,
                   num_devices=NCORES)
    f32 = mybir.dt.float32
    f16 = mybir.dt.float16
    bf16 = mybir.dt.bfloat16
    is_ge = mybir.AluOpType.is_ge
    Sign = mybir.ActivationFunctionType.Sign

    xs_d = nc.dram_tensor("xs", [3, E + mp], bf16, kind="ExternalInput").ap()
    ct_d = nc.dram_tensor("ct", [mp, 130], f32, kind="ExternalInput").ap()
    need_xf = any(r == "gps" for r in bcast_r)
    if need_xf:
        xf_d = nc.dram_tensor("xf", [1, E], f32, kind="ExternalInput").ap()
    out_d = nc.dram_tensor("out", [4, N_LOC], f32, kind="ExternalOutput").ap()

    with tile.TileContext(nc) as tc, ExitStack() as ctx:
        consts = ctx.enter_context(tc.tile_pool(name="consts", bufs=1))
        warmp = ctx.enter_context(tc.tile_pool(name="warm", bufs=1))
        gpb = ctx.enter_context(tc.tile_pool(name="gpb", bufs=2))
        u16p = ctx.enter_context(tc.tile_pool(name="u16", bufs=4))
        outp = ctx.enter_context(tc.tile_pool(name="outp", bufs=1))
        psum = ctx.enter_context(tc.tile_pool(name="psum", bufs=3,
                                              space="PSUM"))
        psacc = ctx.enter_context(tc.tile_pool(name="psacc", bufs=1,
                                               space="PSUM"))
        pswarm = ctx.enter_context(tc.tile_pool(name="pswarm", bufs=1,
                                                space="PSUM"))

        # ---- input DMAs ------------------------------------------------
        # xf first on the Pool engine's own SWDGE queue (it is also the
        # consumer); xs + ct share the SP HWDGE queue.
        if need_xf:
            xf = consts.tile([1, E], f32, tag="xf")
            nc.gpsimd.dma_start(xf[:, :], xf_d)      # SWDGE, Pool engine
        xs = consts.tile([3, E + mp], bf16, tag="xs")
        nc.sync.dma_start(xs[:, :], xs_d)            # SP queue, fastest decode
        ct = consts.tile([mp, 130], f32, tag="ct")
        nc.sync.dma_start(ct[:, :], ct_d)            # SP queue, second

        lpe = ct[:, 0:1]
        nlpe = ct[:, 1:2]
        whi = ct[:, 2:66].bitcast(f16)               # [mp, 128]
        whs = ct[:, 66:130].bitcast(f16)
        ones3 = xs[:, E:E + mp]                      # [3, mp] all-ones bf16

        # ---- warmup: act table load + PE pstate ramp during the DMA head
        wb = warmp.tile([3, 2], bf16, tag="wb")
        nc.gpsimd.memset(wb[:, :], 0.0)
        wf = warmp.tile([1, 2], f32, tag="wf")
        nc.gpsimd.memset(wf[:, :], 0.0)
        wo = warmp.tile([1, 2], f16, tag="wo")
        nc.scalar.activation(wo[:, :], wf[:, :], Sign, bias=0.0, scale=1.0)
        pw = pswarm.tile([2, 2], f32, tag="pw")
        nc.tensor.matmul(pw[:, :], wb[:, 0:2], wb[:, 0:2],
                         start=True, stop=True)

        # ---- pipeline ---------------------------------------------------
        xb_ps = {}                                   # chunk -> PSUM tile
        xb_sb = {}                                   # chunk -> SBUF tile
        u16 = {}
        acc = psacc.tile([4, N_LOC], f32, tag="acc")

        def emit_bcast(k):
            sl = slice(k * CHUNK, (k + 1) * CHUNK)
            if bcast_r[k] == "pe":
                t = psum.tile([mp, CHUNK], f32, tag="xb", name=f"xb{k}")
                # one matmul per 512-col half: PSUM-bank limit
                for h in range(CHUNK // 512):
                    hs = slice(k * CHUNK + h * 512, k * CHUNK + (h + 1) * 512)
                    nc.tensor.matmul(t[:, h * 512:(h + 1) * 512],
                                     ones3[:, :], xs[:, hs],
                                     start=True, stop=True)
                xb_ps[k] = t
            else:
                t = gpb.tile([mp, CHUNK], f32, tag="xg", name=f"xg{k}")
                nc.gpsimd.partition_broadcast(t[:, :], xf[:, sl])
                xb_sb[k] = t

        def emit_cmp(k):
            src = xb_ps[k] if k in xb_ps else xb_sb[k]
            u = u16p.tile([mp, CHUNK], f16, tag="u", name=f"u{k}")
            if cmp_r[k] == "dve":
                nc.vector.tensor_scalar(u[:, :], src[:, :], lpe, None, is_ge)
            else:
                nc.scalar.activation(u[:, :], src[:, :], Sign,
                                     bias=nlpe, scale=1.0)
            u16[k] = u

        acc_started = [False]

        def emit_acc(k, last_chunk):
            tbl = whi if cmp_r[k] == "dve" else whs
            for half in range(FPC):
                f = FPC * k + half
                usl = u16[k][:, half * N_LOC:(half + 1) * N_LOC]
                last = (last_chunk and half == FPC - 1)
                nc.tensor.matmul(acc[:, :], tbl[:, 4 * f:4 * f + 4], usl,
                                 start=not acc_started[0], stop=last)
                acc_started[0] = True

        pe_chunks = [k for k in range(NCHUNK) if bcast_r[k] == "pe"]
        gps_chunks = [k for k in range(NCHUNK) if bcast_r[k] == "gps"]
        acc_seq = _acc_order()
        lookahead = 3

        # gpsimd broadcasts stream independently on the Pool engine
        for k in gps_chunks:
            emit_bcast(k)
        # PE: prefetch broadcasts up to the psum buffer count
        for k in pe_chunks[:lookahead]:
            emit_bcast(k)
        # compares whose broadcast is already emitted, in per-engine chunk
        # order (each engine consumes them in this program order)
        emitted_b = set(gps_chunks) | set(pe_chunks[:lookahead])
        acc_pos = {k: i for i, k in enumerate(acc_seq)}
        for k in sorted(emitted_b, key=lambda k: acc_pos[k]):
            emit_cmp(k)
        # accumulates in ACC_ORDER; late broadcasts (+their compares)
        # interleave into the stream as buffers free up
        nxt = lookahead
        for i, k in enumerate(acc_seq):
            emit_acc(k, last_chunk=(i == NCHUNK - 1))
            if nxt < len(pe_chunks):
                kb = pe_chunks[nxt]
                emit_bcast(kb)
                emit_cmp(kb)
                nxt += 1

        # ---- output -----------------------------------------------------
        outs = outp.tile([4, N_LOC], f32)
        nc.vector.tensor_scalar(outs[:, :], acc[:, :], 0.0, None,
                                mybir.AluOpType.add)
        nc.sync.dma_start(out_d, outs[:, :])

    nc.compile()
    return nc


# ----------------------------------------------------------------------------
# Entry point
# ----------------------------------------------------------------------------

def kernel(x, initial_cond, threshold, epsilon, W, b):
    global LAST_RESULTS, LAST_NC
    x = np.ascontiguousarray(np.asarray(x, np.float32))
    W = np.asarray(W, np.float32)
    b = np.asarray(b, np.float32)
    ic = float(np.asarray(initial_cond).reshape(-1)[0])
    thr = float(np.asarray(threshold).reshape(-1)[0])
    eps = float(np.asarray(epsilon).reshape(-1)[0])

    consts, whi, whs, mp, K, corrections, bcast_r, cmp_r = _build_tables(
        x, ic, thr, eps, W, b)
    need_xf = any(r == "gps" for r in bcast_r)

    nc = _build_device_program(mp, bcast_r, cmp_r)
    LAST_NC = nc

    bf = ml_dtypes.bfloat16
    in_maps = []
    for d in range(NCORES):
        xd = x[d * N_LOC:(d + 1) * N_LOC, :]         # [256, 32]
        xrow = np.ascontiguousarray(xd.T).reshape(E)  # f-major
        hi, mid, lo = _split_bf16_3(xrow)
        xsplit = np.ones((3, E + mp), bf)
        xsplit[0, :E] = hi
        xsplit[1, :E] = mid
        xsplit[2, :E] = lo
        im = {"xs": xsplit, "ct": consts}
        if need_xf:
            im["xf"] = xrow.reshape(1, E)
        in_maps.append(im)

    res = run_bass_kernel_spmd(nc, in_maps, core_ids=list(range(NCORES)))
    LAST_RESULTS = res

    out = np.empty((N, 2), np.float64)
    for d in range(NCORES):
        o4 = res.results[d]["out"].astype(np.float64)  # [4, 256] hi/lo rows
        out[d * N_LOC:(d + 1) * N_LOC, :] = (o4[:2] + o4[2:]).T
    out += b.astype(np.float64).reshape(1, 2) + K.reshape(1, 2)
    for (n, f, m) in corrections:
        for c in range(2):
            out[n, c] += (float(whs[m, 4 * f + c])
                          + float(whs[m, 4 * f + 2 + c]))
    return out.astype(np.float32)


# revision 22
# speedup vs baseline: 1.2115x; 1.1378x over previous
"""ChaosNet (ChaosFEX + linear head) Trainium2 kernel.

Math restructure: every per-element feature depends only on k*(x) = first
trajectory index k with |traj[k] - x| < eps.  k*(x) is piecewise-constant in x
(first-claim intervals of the shared trajectory), so the model output

    out[n, c] = b_c + sum_f Phi_{c,f}(k*(x[n,f]))

is, per (c, f), a piecewise-constant function of x with M segments.  With
region left-edges L_0 <= ... <= L_{M-1} and per-segment table values Phi[m],
a telescoped form needs only rank indicators:

    Phi(x) = sum_m [x >= L_m] * dPhi[m]          (dPhi = successive deltas)

Device pipeline (per core, 256 rows of x, E = 8192 elements, mp regions in
the partition dim):
  - x is broadcast across the mp partitions in 512-column chunks, two ways:
      * PE: one ones-matmul per chunk over an exact 3-way bf16 split of x
        (x = hi+mid+lo exactly; the [3,mp] all-ones stationary reproduces x
        bit-exactly in PSUM at 1 PE cycle/column), or
      * gpsimd partition_broadcast from an f-major fp32 row (slower per
        column but runs on an otherwise idle engine).
  - compares u[m, j] = [x_j >= L_m] run on two engines:
      * DVE tensor_scalar is_ge -> fp16 {0,1}  (exact), or
      * Activation Sign(x - L) -> fp16 {-1,0,+1}; the (s+1)/2 re-encoding is
        folded into 0.5-scaled weight tables plus a per-channel constant,
        and the (measure-zero) x == L exact hits are patched on the host.
  - PE accumulates out[c, n] += sum_m u[m, f*256+n] * W'[m, 2f+c] over all 32
    f-blocks into one [4, 256] PSUM tile (fp16 hi/lo weight pairs).
  - DVE copies PSUM -> SBUF, one DMA out.

The host does only the inherently sequential scalar work: the 10000-step
trajectory, its prefix sums, and the exact-fp32 region partition (binary
search on fp32 bit patterns, so region edges reproduce the reference's
fp32 comparison semantics exactly).
"""

import os
import sys
from contextlib import ExitStack

import ml_dtypes
import numpy as np

sys.path.insert(0, "/opt/trn_rl_repo")

import concourse.bass as bass  # noqa: E402
import concourse.tile as tile  # noqa: E402
from concourse import bacc, mybir  # noqa: E402
from concourse.bass_utils import run_bass_kernel_spmd  # noqa: E402

T = 10000
N = 2048
F = 32
NCORES = 8
N_LOC = N // NCORES            # 256 rows per core
E = N_LOC * F                  # 8192 elements per core (f-major columns)
CHUNK = 1024                   # columns per pipeline chunk (= 4 f-blocks)
NCHUNK = E // CHUNK            # 8
FPC = CHUNK // N_LOC           # f-blocks per chunk (4)

np.seterr(all="ignore")

LAST_RESULTS = None            # BassKernelResults of the most recent run
LAST_NC = None                 # compiled Bass program of the most recent run


# ----------------------------------------------------------------------------
# Host-side preprocessing
# ----------------------------------------------------------------------------

def _build_traj(ic, thr):
    """fp32 skew-tent trajectory, bit-identical to the jax scan."""
    traj = np.empty(T, np.float32)
    z = np.float32(ic)
    thr = np.float32(thr)
    one = np.float32(1.0)
    omt = np.float32(one - thr)
    for k in range(T):
        traj[k] = z
        z = np.float32(z / thr) if z < thr else np.float32((one - z) / omt)
    return traj


def _sortable(i):
    """int32 bit pattern -> order-isomorphic int32 key (handles negatives)."""
    return np.where(i >= 0, i, i ^ np.int32(0x7FFFFFFF))


def _unsortable(k):
    return np.where(k >= 0, k, k ^ np.int32(0x7FFFFFFF))


def _match_intervals(traj, eps, xmin, xmax):
    """Exact fp32 interval [lo_k, hi_k] of {x in [xmin,xmax] :
    |fl32(traj_k - x)| < eps}; valid[k]=False if empty."""
    eps = np.float32(eps)
    xmin = np.float32(xmin)
    xmax = np.float32(xmax)

    def cond(xs):
        return np.abs(traj - xs.astype(np.float32)) < eps

    anchor = np.clip(traj, xmin, xmax)
    valid = cond(anchor)

    I = lambda f: _sortable(f.view(np.int32))             # noqa: E731
    Fv = lambda k: _unsortable(k).view(np.float32)        # noqa: E731

    def bisect(lo_i, hi_i, need, want_smallest_true):
        # invariant: cond(Fv(hi_i)) True/False per direction; int keys.
        for _ in range(40):
            gap = np.where(need, hi_i - lo_i, 0)
            if (gap <= 1).all():
                break
            mid = ((lo_i.astype(np.int64) + hi_i) // 2).astype(np.int32)
            cm = cond(Fv(mid))
            if want_smallest_true:
                hi_i = np.where(need & cm, mid, hi_i)
                lo_i = np.where(need & ~cm, mid, lo_i)
            else:
                lo_i = np.where(need & cm, mid, lo_i)
                hi_i = np.where(need & ~cm, mid, hi_i)
        return lo_i, hi_i

    # left edge: smallest x in [xmin, anchor] with cond True
    at_min = cond(np.full(T, xmin, np.float32))
    lo_edge = np.where(at_min, xmin, np.float32(np.nan))
    need = valid & np.isnan(lo_edge)
    lo_i = np.broadcast_to(I(xmin.reshape(1)), (T,)).copy()
    hi_i = I(anchor.copy())
    lo_i, hi_i = bisect(lo_i, hi_i, need, True)
    lo_edge = np.where(np.isnan(lo_edge), Fv(hi_i), lo_edge)

    # right edge: largest x in [anchor, xmax] with cond True
    at_max = cond(np.full(T, xmax, np.float32))
    hi_edge = np.where(at_max, xmax, np.float32(np.nan))
    need = valid & np.isnan(hi_edge)
    lo_i = I(anchor.copy())
    hi_i = np.broadcast_to(I(xmax.reshape(1)), (T,)).copy()
    lo_i, hi_i = bisect(lo_i, hi_i, need, False)
    hi_edge = np.where(np.isnan(hi_edge), Fv(lo_i), hi_edge)

    # exactness checks (cheap, vectorized)
    v = valid
    assert cond(np.where(v, lo_edge, anchor)).all()
    assert cond(np.where(v, hi_edge, anchor)).all()
    below = np.nextafter(lo_edge, np.float32(-np.inf))
    above = np.nextafter(hi_edge, np.float32(np.inf))
    assert not (v & (below >= xmin) & cond(below)).any()
    assert not (v & (above <= xmax) & cond(above)).any()
    return lo_edge, hi_edge, valid


def _build_regions(traj, eps, xmin, xmax):
    """First-claim partition of [xmin, xmax] into regions of constant k*.
    Returns sorted left edges L (fp32) and per-region kstar (== T: never)."""
    xl, xr, valid = _match_intervals(traj, eps, xmin, xmax)
    down = lambda a: np.nextafter(a, np.float32(-np.inf))  # noqa: E731
    up = lambda a: np.nextafter(a, np.float32(np.inf))     # noqa: E731
    uncovered = [(np.float32(xmin), np.float32(xmax))]
    regions = []
    for k in range(T):
        if not uncovered:
            break
        if not valid[k]:
            continue
        lo_k, hi_k = xl[k], xr[k]
        new_unc = []
        for (a, b) in uncovered:
            if lo_k > b or hi_k < a:
                new_unc.append((a, b))
                continue
            ra, rb = max(lo_k, a), min(hi_k, b)
            regions.append((ra, k))
            if a < ra:
                new_unc.append((a, down(ra)))
            if rb < b:
                new_unc.append((up(rb), b))
        uncovered = new_unc
    for (a, b) in uncovered:
        regions.append((a, T))
    regions.sort(key=lambda r: r[0])
    L = np.array([r[0] for r in regions], np.float32)
    ks = np.array([r[1] for r in regions], np.int64)
    return L, ks


def _region_features(traj, thr, ks):
    """Per-region (tt, energy, p, ent) with the reference's fp32 accumulation
    semantics (sequential fp32 cumsum == per-step fp32 adds)."""
    thr = np.float32(thr)
    t2 = traj * traj                                  # fp32 squares
    Ecum = np.cumsum(t2, dtype=np.float32)            # sequential fp32 adds
    gt = (traj > thr).astype(np.float32)
    Ccum = np.cumsum(gt, dtype=np.float32)            # exact small ints
    fired = ks < T
    j = np.where(fired, ks, T - 1)
    tt = np.where(fired, ks + 1, T).astype(np.float32)
    en = Ecum[j].astype(np.float32)
    cnt = Ccum[j].astype(np.float32)
    p = (cnt / tt).astype(np.float32)

    def xlog2x(v):
        safe = np.where(v > 0, v, np.float32(1.0)).astype(np.float32)
        return np.where(v > 0, v * np.log2(safe, dtype=np.float32),
                        np.float32(0.0)).astype(np.float32)

    ent = -(xlog2x(p) + xlog2x((np.float32(1.0) - p).astype(np.float32)))
    return tt, en, p, ent.astype(np.float32)


def _split_bf16_3(x32):
    """Exact 3-way bf16 split: x == hi + mid + lo (verified)."""
    bf = ml_dtypes.bfloat16
    hi = x32.astype(bf)
    r1 = (x32 - hi.astype(np.float32)).astype(np.float32)
    mid = r1.astype(bf)
    r2 = (r1 - mid.astype(np.float32)).astype(np.float32)
    lo = r2.astype(bf)
    recon = ((hi.astype(np.float32) + mid.astype(np.float32))
             + lo.astype(np.float32)).astype(np.float32)
    assert np.array_equal(recon, x32), "3-way bf16 split is not exact"
    recon2 = (hi.astype(np.float32)
              + (mid.astype(np.float32) + lo.astype(np.float32)))
    assert np.array_equal(recon2.astype(np.float32), x32), \
        "3-way bf16 split order-sensitive"
    return hi, mid, lo


# Per-chunk routing.  BCAST[k] in {"pe", "gps"}; CMP[k] in {"dve", "act"}.
# "act" chunks use the Sign encoding (0.5-scaled tables + constant).
def _routes():
    gps = os.environ.get("GPS_CHUNKS", "5,6,7")
    act = os.environ.get("ACT_CHUNKS", "0,2,3,4")
    gps = set(int(s) for s in gps.split(",") if s != "")
    act = set(int(s) for s in act.split(",") if s != "")
    bcast = ["gps" if k in gps else "pe" for k in range(NCHUNK)]
    cmp_ = ["act" if k in act else "dve" for k in range(NCHUNK)]
    return bcast, cmp_


def _acc_order():
    """PE accumulation order: gate the tail on early-completing compares."""
    s = os.environ.get("ACC_ORDER", "0,5,1,2,6,3,7,4")
    order = [int(v) for v in s.split(",")]
    assert sorted(order) == list(range(NCHUNK))
    return order


def _build_tables(x, ic, thr, eps, W, b):
    """Builds all device-side tables plus host-side output corrections."""
    traj = _build_traj(ic, thr)
    L, ks = _build_regions(traj, eps, float(x.min()), float(x.max()))
    tt, en, p, ent = _region_features(traj, thr, ks)
    M = L.shape[0]
    assert M <= 128, f"region count {M} exceeds one partition block"

    # Phi[m, 2f+c] = W[c,4f]*tt + W[c,4f+1]*en + W[c,4f+2]*p + W[c,4f+3]*ent
    W64 = W.astype(np.float64).reshape(2, F, 4)
    feats64 = np.stack([tt, en, p, ent], -1).astype(np.float64)   # [M, 4]
    phi = np.einsum("mj,cfj->mcf", feats64, W64)                  # [M, 2, F]
    phi = phi.transpose(0, 2, 1).reshape(M, 2 * F)                # [M, 64]

    # compensated fp32 deltas: partial fp32 sums track the fp64 table
    dphi = np.empty((M, 2 * F), np.float32)
    running = np.zeros(2 * F, np.float64)
    for m in range(M):
        d = (phi[m] - running).astype(np.float32)
        dphi[m] = d
        running += d.astype(np.float64)

    # pad M to a multiple of 8 partitions; L pad = +inf (never <= x)
    mp = max(16, ((M + 7) // 8) * 8)
    L_pad = np.full(mp, np.float32(np.inf), np.float32)
    L_pad[:M] = L
    dphi_pad = np.zeros((mp, 2 * F), np.float32)
    dphi_pad[:M] = dphi

    def pack_hilo(d32):
        """[mp, 2F] fp32 -> [mp, 4F] fp16: per f (hi_c0, hi_c1, lo_c0, lo_c1)."""
        hi16 = d32.astype(np.float16)
        lo16 = (d32.astype(np.float64) - hi16.astype(np.float64)) \
            .astype(np.float16)
        out = np.empty((mp, 4 * F), np.float16)
        for f in range(F):
            out[:, 4 * f:4 * f + 2] = hi16[:, 2 * f:2 * f + 2]
            out[:, 4 * f + 2:4 * f + 4] = lo16[:, 2 * f:2 * f + 2]
        return out

    whi = pack_hilo(dphi_pad)                        # is_ge chunks
    whs = pack_hilo(0.5 * dphi_pad)                  # Sign chunks (0.5-scaled)

    # consts [mp, 130] fp32: col0 = L, col1 = -L, cols 2:66 = whi (f16 pairs
    # viewed as f32 words), cols 66:130 = whs
    consts = np.zeros((mp, 130), np.float32)
    consts[:, 0] = L_pad
    consts[:, 1] = -L_pad
    consts[:, 2:66] = whi.view(np.float32)
    consts[:, 66:130] = whs.view(np.float32)

    bcast_r, cmp_r = _routes()
    # Sign-path constant per channel: K_c = sum over sign-chunk features f of
    # sum_m [(0.5 d)_hi + (0.5 d)_lo]  (from the actual device fp16 tables)
    K = np.zeros(2, np.float64)
    sign_f = [f for f in range(F) if cmp_r[f // FPC] == "act"]
    for f in sign_f:
        for c in range(2):
            K[c] += (whs[:, 4 * f + c].astype(np.float64).sum()
                     + whs[:, 4 * f + 2 + c].astype(np.float64).sum())

    # exact x == L hits on Sign-path features lose 0.5*dphi (sign(0) = 0)
    corrections = []                                 # (n, f, m) triples
    hit_rows, hit_fs = np.nonzero(np.isin(x, L[:M]))
    for n, f in zip(hit_rows, hit_fs):
        if cmp_r[f // FPC] != "act":
            continue
        m = int(np.nonzero(L[:M] == x[n, f])[0][0])
        corrections.append((int(n), int(f), m))

    return consts, whi, whs, mp, K, corrections, bcast_r, cmp_r


# ----------------------------------------------------------------------------
# Device kernel
# ----------------------------------------------------------------------------

def _build_device_program(mp, bcast_r, cmp_r):
    nc = bacc.Bacc("TRN2", target_bir_lowering=False, debug=False,
                   num_devices=NCORES)
    f32 = mybir.dt.float32
    f16 = mybir.dt.float16
    bf16 = mybir.dt.bfloat16
    is_ge = mybir.AluOpType.is_ge
    Sign = mybir.ActivationFunctionType.Sign

    xs_d = nc.dram_tensor("xs", [3, E + mp], bf16, kind="ExternalInput").ap()
    ct_d = nc.dram_tensor("ct", [mp, 130], f32, kind="ExternalInput").ap()
    need_xf = any(r == "gps" for r in bcast_r)
    if need_xf:
        xf_d = nc.dram_tensor("xf", [1, E], f32, kind="ExternalInput").ap()
    out_d = nc.dram_tensor("out", [4, N_LOC], f32, kind="ExternalOutput").ap()

    with tile.TileContext(nc) as tc, ExitStack() as ctx:
        consts = ctx.enter_context(tc.tile_pool(name="consts", bufs=1))
        warmp = ctx.enter_context(tc.tile_pool(name="warm", bufs=1))
        gpb = ctx.enter_context(tc.tile_pool(name="gpb", bufs=2))
        u16p = ctx.enter_context(tc.tile_pool(name="u16", bufs=8))
        outp = ctx.enter_context(tc.tile_pool(name="outp", bufs=1))
        psum = ctx.enter_context(tc.tile_pool(name="psum", bufs=3,
                                              space="PSUM"))
        psacc = ctx.enter_context(tc.tile_pool(name="psacc", bufs=1,
                                               space="PSUM"))
        pswarm = ctx.enter_context(tc.tile_pool(name="pswarm", bufs=1,
                                                space="PSUM"))

        # ---- input DMAs ------------------------------------------------
        # xf first on the Pool engine's own SWDGE queue (it is also the
        # consumer); xs + ct share the SP HWDGE queue.
        if need_xf:
            xf = consts.tile([1, E], f32, tag="xf")
            nc.gpsimd.dma_start(xf[:, :], xf_d)      # SWDGE, Pool engine
        xs = consts.tile([3, E + mp], bf16, tag="xs")
        nc.sync.dma_start(xs[:, :], xs_d)            # SP queue, fastest decode
        ct = consts.tile([mp, 130], f32, tag="ct")
        nc.sync.dma_start(ct[:, :], ct_d)            # SP queue, second

        lpe = ct[:, 0:1]
        nlpe = ct[:, 1:2]
        whi = ct[:, 2:66].bitcast(f16)               # [mp, 128]
        whs = ct[:, 66:130].bitcast(f16)
        ones3 = xs[:, E:E + mp]                      # [3, mp] all-ones bf16

        # ---- warmup: act table load + PE pstate ramp during the DMA head.
        # Const APs are dependency-free (initialized behind the program's
        # startup barrier), so these run immediately.
        cf0 = nc.const_aps.aps[(f32, 0.0)][0:1, 0:1]
        cb1 = nc.const_aps.aps[(bf16, 1.0)][0:1, 0:1]
        wo = warmp.tile([1, 1], f16, tag="wo")
        nc.scalar.activation(wo[:, :], cf0, Sign, bias=0.0, scale=1.0)
        pw = pswarm.tile([1, 1], f32, tag="pw")
        nc.tensor.matmul(pw[:, :], cb1, cb1, start=True, stop=True)

        # ---- pipeline ---------------------------------------------------
        xb_ps = {}                                   # chunk -> PSUM tile
        xb_sb = {}                                   # chunk -> SBUF tile
        u16 = {}
        acc = psacc.tile([4, N_LOC], f32, tag="acc")

        def emit_bcast(k):
            sl = slice(k * CHUNK, (k + 1) * CHUNK)
            if bcast_r[k] == "pe":
                t = psum.tile([mp, CHUNK], f32, tag="xb", name=f"xb{k}")
                # one matmul per 512-col half: PSUM-bank limit
                for h in range(CHUNK // 512):
                    hs = slice(k * CHUNK + h * 512, k * CHUNK + (h + 1) * 512)
                    nc.tensor.matmul(t[:, h * 512:(h + 1) * 512],
                                     ones3[:, :], xs[:, hs],
                                     start=True, stop=True)
                xb_ps[k] = t
            else:
                t = gpb.tile([mp, CHUNK], f32, tag="xg", name=f"xg{k}")
                nc.gpsimd.partition_broadcast(t[:, :], xf[:, sl])
                xb_sb[k] = t

        def emit_cmp(k):
            src = xb_ps[k] if k in xb_ps else xb_sb[k]
            u = u16p.tile([mp, CHUNK], f16, tag="u", name=f"u{k}")
            if cmp_r[k] == "dve":
                nc.vector.tensor_scalar(u[:, :], src[:, :], lpe, None, is_ge)
            else:
                nc.scalar.activation(u[:, :], src[:, :], Sign,
                                     bias=nlpe, scale=1.0)
            u16[k] = u

        acc_started = [False]

        def emit_acc(k, last_chunk):
            tbl = whi if cmp_r[k] == "dve" else whs
            for half in range(FPC):
                f = FPC * k + half
                usl = u16[k][:, half * N_LOC:(half + 1) * N_LOC]
                last = (last_chunk and half == FPC - 1)
                nc.tensor.matmul(acc[:, :], tbl[:, 4 * f:4 * f + 4], usl,
                                 start=not acc_started[0], stop=last)
                acc_started[0] = True

        pe_chunks = [k for k in range(NCHUNK) if bcast_r[k] == "pe"]
        gps_chunks = [k for k in range(NCHUNK) if bcast_r[k] == "gps"]
        acc_seq = _acc_order()
        lookahead = 3

        # gpsimd broadcasts stream independently on the Pool engine
        for k in gps_chunks:
            emit_bcast(k)
        # PE: prefetch broadcasts up to the psum buffer count
        for k in pe_chunks[:lookahead]:
            emit_bcast(k)
        # compares whose broadcast is already emitted, in per-engine chunk
        # order (each engine consumes them in this program order)
        emitted_b = set(gps_chunks) | set(pe_chunks[:lookahead])
        acc_pos = {k: i for i, k in enumerate(acc_seq)}
        for k in sorted(emitted_b, key=lambda k: acc_pos[k]):
            emit_cmp(k)
        # accumulates in ACC_ORDER; late broadcasts (+their compares)
        # interleave into the stream as buffers free up
        nxt = lookahead
        for i, k in enumerate(acc_seq):
            emit_acc(k, last_chunk=(i == NCHUNK - 1))
            if nxt < len(pe_chunks):
                kb = pe_chunks[nxt]
                emit_bcast(kb)
                emit_cmp(kb)
                nxt += 1

        # ---- output -----------------------------------------------------
        outs = outp.tile([4, N_LOC], f32)
        nc.vector.tensor_scalar(outs[:, :], acc[:, :], 0.0, None,
                                mybir.AluOpType.add)
        nc.sync.dma_start(out_d, outs[:, :])

    nc.compile()
    return nc


# ----------------------------------------------------------------------------
# Entry point
# ----------------------------------------------------------------------------

def kernel(x, initial_cond, threshold, epsilon, W, b):
    global LAST_RESULTS, LAST_NC
    x = np.ascontiguousarray(np.asarray(x, np.float32))
    W = np.asarray(W, np.float32)
    b = np.asarray(b, np.float32)
    ic = float(np.asarray(initial_cond).reshape(-1)[0])
    thr = float(np.asarray(threshold).reshape(-1)[0])
    eps = float(np.asarray(epsilon).reshape(-1)[0])

    consts, whi, whs, mp, K, corrections, bcast_r, cmp_r = _build_tables(
        x, ic, thr, eps, W, b)
    need_xf = any(r == "gps" for r in bcast_r)

    nc = _build_device_program(mp, bcast_r, cmp_r)
    LAST_NC = nc

    bf = ml_dtypes.bfloat16
    in_maps = []
    for d in range(NCORES):
        xd = x[d * N_LOC:(d + 1) * N_LOC, :]         # [256, 32]
        xrow = np.ascontiguousarray(xd.T).reshape(E)  # f-major
        hi, mid, lo = _split_bf16_3(xrow)
        xsplit = np.ones((3, E + mp), bf)
        xsplit[0, :E] = hi
        xsplit[1, :E] = mid
        xsplit[2, :E] = lo
        im = {"xs": xsplit, "ct": consts}
        if need_xf:
            im["xf"] = xrow.reshape(1, E)
        in_maps.append(im)

    res = run_bass_kernel_spmd(nc, in_maps, core_ids=list(range(NCORES)))
    LAST_RESULTS = res

    out = np.empty((N, 2), np.float64)
    for d in range(NCORES):
        o4 = res.results[d]["out"].astype(np.float64)  # [4, 256] hi/lo rows
        out[d * N_LOC:(d + 1) * N_LOC, :] = (o4[:2] + o4[2:]).T
    out += b.astype(np.float64).reshape(1, 2) + K.reshape(1, 2)
    for (n, f, m) in corrections:
        for c in range(2):
            out[n, c] += (float(whs[m, 4 * f + c])
                          + float(whs[m, 4 * f + 2 + c]))
    return out.astype(np.float32)


# revision 39
# speedup vs baseline: 1.3187x; 1.0885x over previous
"""ChaosNet (ChaosFEX + linear head) Trainium2 kernel.

Math restructure: every per-element feature depends only on k*(x) = first
trajectory index k with |traj[k] - x| < eps.  k*(x) is piecewise-constant in x
(first-claim intervals of the shared trajectory), so the model output

    out[n, c] = b_c + sum_f Phi_{c,f}(k*(x[n,f]))

is, per (c, f), a piecewise-constant function of x with M segments.  With
region left-edges L_0 <= ... <= L_{M-1} and per-segment table values Phi[m],
a telescoped form needs only rank indicators:

    Phi(x) = sum_m [x >= L_m] * dPhi[m]          (dPhi = successive deltas)

Device pipeline (per core, 256 rows of x, E = 8192 elements, mp regions in
the partition dim):
  - x is broadcast across the mp partitions in 512-column chunks, two ways:
      * PE: one ones-matmul per chunk over an exact 3-way bf16 split of x
        (x = hi+mid+lo exactly; the [3,mp] all-ones stationary reproduces x
        bit-exactly in PSUM at 1 PE cycle/column), or
      * gpsimd partition_broadcast from an f-major fp32 row (slower per
        column but runs on an otherwise idle engine).
  - compares u[m, j] = [x_j >= L_m] run on two engines:
      * DVE tensor_scalar is_ge -> fp16 {0,1}  (exact), or
      * Activation Sign(x - L) -> fp16 {-1,0,+1}; the (s+1)/2 re-encoding is
        folded into 0.5-scaled weight tables plus a per-channel constant,
        and the (measure-zero) x == L exact hits are patched on the host.
  - PE accumulates out[c, n] += sum_m u[m, f*256+n] * W'[m, 2f+c] over all 32
    f-blocks into one [4, 256] PSUM tile (fp16 hi/lo weight pairs).
  - DVE copies PSUM -> SBUF, one DMA out.

The host does only the inherently sequential scalar work: the 10000-step
trajectory, its prefix sums, and the exact-fp32 region partition (binary
search on fp32 bit patterns, so region edges reproduce the reference's
fp32 comparison semantics exactly).
"""

import os
import sys
from contextlib import ExitStack

import ml_dtypes
import numpy as np

sys.path.insert(0, "/opt/trn_rl_repo")

import concourse.bass as bass  # noqa: E402
import concourse.tile as tile  # noqa: E402
from concourse import bacc, mybir  # noqa: E402
from concourse.bass_utils import run_bass_kernel_spmd  # noqa: E402

T = 10000
N = 2048
F = 32
NCORES = 8
N_LOC = N // NCORES            # 256 rows per core
E = N_LOC * F                  # 8192 elements per core (f-major columns)
CHUNK = 1024                   # columns per pipeline chunk (= 4 f-blocks)
NCHUNK = E // CHUNK            # 8
FPC = CHUNK // N_LOC           # f-blocks per chunk (4)

np.seterr(all="ignore")

LAST_RESULTS = None            # BassKernelResults of the most recent run
LAST_NC = None                 # compiled Bass program of the most recent run


# ----------------------------------------------------------------------------
# Host-side preprocessing
# ----------------------------------------------------------------------------

def _build_traj(ic, thr):
    """fp32 skew-tent trajectory, bit-identical to the jax scan."""
    traj = np.empty(T, np.float32)
    z = np.float32(ic)
    thr = np.float32(thr)
    one = np.float32(1.0)
    omt = np.float32(one - thr)
    for k in range(T):
        traj[k] = z
        z = np.float32(z / thr) if z < thr else np.float32((one - z) / omt)
    return traj


def _sortable(i):
    """int32 bit pattern -> order-isomorphic int32 key (handles negatives)."""
    return np.where(i >= 0, i, i ^ np.int32(0x7FFFFFFF))


def _unsortable(k):
    return np.where(k >= 0, k, k ^ np.int32(0x7FFFFFFF))


def _match_intervals(traj, eps, xmin, xmax):
    """Exact fp32 interval [lo_k, hi_k] of {x in [xmin,xmax] :
    |fl32(traj_k - x)| < eps}; valid[k]=False if empty."""
    eps = np.float32(eps)
    xmin = np.float32(xmin)
    xmax = np.float32(xmax)

    def cond(xs):
        return np.abs(traj - xs.astype(np.float32)) < eps

    anchor = np.clip(traj, xmin, xmax)
    valid = cond(anchor)

    I = lambda f: _sortable(f.view(np.int32))             # noqa: E731
    Fv = lambda k: _unsortable(k).view(np.float32)        # noqa: E731

    def bisect(lo_i, hi_i, need, want_smallest_true):
        # invariant: cond(Fv(hi_i)) True/False per direction; int keys.
        for _ in range(40):
            gap = np.where(need, hi_i - lo_i, 0)
            if (gap <= 1).all():
                break
            mid = ((lo_i.astype(np.int64) + hi_i) // 2).astype(np.int32)
            cm = cond(Fv(mid))
            if want_smallest_true:
                hi_i = np.where(need & cm, mid, hi_i)
                lo_i = np.where(need & ~cm, mid, lo_i)
            else:
                lo_i = np.where(need & cm, mid, lo_i)
                hi_i = np.where(need & ~cm, mid, hi_i)
        return lo_i, hi_i

    # left edge: smallest x in [xmin, anchor] with cond True
    at_min = cond(np.full(T, xmin, np.float32))
    lo_edge = np.where(at_min, xmin, np.float32(np.nan))
    need = valid & np.isnan(lo_edge)
    lo_i = np.broadcast_to(I(xmin.reshape(1)), (T,)).copy()
    hi_i = I(anchor.copy())
    lo_i, hi_i = bisect(lo_i, hi_i, need, True)
    lo_edge = np.where(np.isnan(lo_edge), Fv(hi_i), lo_edge)

    # right edge: largest x in [anchor, xmax] with cond True
    at_max = cond(np.full(T, xmax, np.float32))
    hi_edge = np.where(at_max, xmax, np.float32(np.nan))
    need = valid & np.isnan(hi_edge)
    lo_i = I(anchor.copy())
    hi_i = np.broadcast_to(I(xmax.reshape(1)), (T,)).copy()
    lo_i, hi_i = bisect(lo_i, hi_i, need, False)
    hi_edge = np.where(np.isnan(hi_edge), Fv(lo_i), hi_edge)

    # exactness checks (cheap, vectorized)
    v = valid
    assert cond(np.where(v, lo_edge, anchor)).all()
    assert cond(np.where(v, hi_edge, anchor)).all()
    below = np.nextafter(lo_edge, np.float32(-np.inf))
    above = np.nextafter(hi_edge, np.float32(np.inf))
    assert not (v & (below >= xmin) & cond(below)).any()
    assert not (v & (above <= xmax) & cond(above)).any()
    return lo_edge, hi_edge, valid


def _build_regions(traj, eps, xmin, xmax):
    """First-claim partition of [xmin, xmax] into regions of constant k*.
    Returns sorted left edges L (fp32) and per-region kstar (== T: never)."""
    xl, xr, valid = _match_intervals(traj, eps, xmin, xmax)
    down = lambda a: np.nextafter(a, np.float32(-np.inf))  # noqa: E731
    up = lambda a: np.nextafter(a, np.float32(np.inf))     # noqa: E731
    uncovered = [(np.float32(xmin), np.float32(xmax))]
    regions = []
    for k in range(T):
        if not uncovered:
            break
        if not valid[k]:
            continue
        lo_k, hi_k = xl[k], xr[k]
        new_unc = []
        for (a, b) in uncovered:
            if lo_k > b or hi_k < a:
                new_unc.append((a, b))
                continue
            ra, rb = max(lo_k, a), min(hi_k, b)
            regions.append((ra, k))
            if a < ra:
                new_unc.append((a, down(ra)))
            if rb < b:
                new_unc.append((up(rb), b))
        uncovered = new_unc
    for (a, b) in uncovered:
        regions.append((a, T))
    regions.sort(key=lambda r: r[0])
    L = np.array([r[0] for r in regions], np.float32)
    ks = np.array([r[1] for r in regions], np.int64)
    return L, ks


def _region_features(traj, thr, ks):
    """Per-region (tt, energy, p, ent) with the reference's fp32 accumulation
    semantics (sequential fp32 cumsum == per-step fp32 adds)."""
    thr = np.float32(thr)
    t2 = traj * traj                                  # fp32 squares
    Ecum = np.cumsum(t2, dtype=np.float32)            # sequential fp32 adds
    gt = (traj > thr).astype(np.float32)
    Ccum = np.cumsum(gt, dtype=np.float32)            # exact small ints
    fired = ks < T
    j = np.where(fired, ks, T - 1)
    tt = np.where(fired, ks + 1, T).astype(np.float32)
    en = Ecum[j].astype(np.float32)
    cnt = Ccum[j].astype(np.float32)
    p = (cnt / tt).astype(np.float32)

    def xlog2x(v):
        safe = np.where(v > 0, v, np.float32(1.0)).astype(np.float32)
        return np.where(v > 0, v * np.log2(safe, dtype=np.float32),
                        np.float32(0.0)).astype(np.float32)

    ent = -(xlog2x(p) + xlog2x((np.float32(1.0) - p).astype(np.float32)))
    return tt, en, p, ent.astype(np.float32)


def _split_bf16_3(x32):
    """Exact 3-way bf16 split: x == hi + mid + lo (verified)."""
    bf = ml_dtypes.bfloat16
    hi = x32.astype(bf)
    r1 = (x32 - hi.astype(np.float32)).astype(np.float32)
    mid = r1.astype(bf)
    r2 = (r1 - mid.astype(np.float32)).astype(np.float32)
    lo = r2.astype(bf)
    recon = ((hi.astype(np.float32) + mid.astype(np.float32))
             + lo.astype(np.float32)).astype(np.float32)
    assert np.array_equal(recon, x32), "3-way bf16 split is not exact"
    recon2 = (hi.astype(np.float32)
              + (mid.astype(np.float32) + lo.astype(np.float32)))
    assert np.array_equal(recon2.astype(np.float32), x32), \
        "3-way bf16 split order-sensitive"
    return hi, mid, lo


# Per-chunk routing.  BCAST[k] in {"pe", "gps"}; CMP[k] in {"dve", "act"}.
# "act" chunks use the Sign encoding (0.5-scaled tables + constant).
def _routes():
    gps = os.environ.get("GPS_CHUNKS", "6,7")
    act = os.environ.get("ACT_CHUNKS", "0,2,3,4")
    pool = os.environ.get("POOL_CMPS", "")
    gps = set(int(s) for s in gps.split(",") if s != "")
    act = set(int(s) for s in act.split(",") if s != "")
    pool = set(int(s) for s in pool.split(",") if s != "")
    bcast = ["gps" if k in gps else "pe" for k in range(NCHUNK)]
    cmp_ = ["act" if k in act else ("pool" if k in pool else "dve")
            for k in range(NCHUNK)]
    return bcast, cmp_


def _acc_order():
    """PE accumulation order: gate the tail on early-completing compares."""
    s = os.environ.get("ACC_ORDER", "0,1,2,6,3,7,5,4")
    order = [int(v) for v in s.split(",")]
    assert sorted(order) == list(range(NCHUNK))
    return order


def _build_tables(x, ic, thr, eps, W, b):
    """Builds all device-side tables plus host-side output corrections."""
    traj = _build_traj(ic, thr)
    L, ks = _build_regions(traj, eps, float(x.min()), float(x.max()))
    tt, en, p, ent = _region_features(traj, thr, ks)
    M = L.shape[0]
    assert M <= 128, f"region count {M} exceeds one partition block"

    # Phi[m, 2f+c] = W[c,4f]*tt + W[c,4f+1]*en + W[c,4f+2]*p + W[c,4f+3]*ent
    W64 = W.astype(np.float64).reshape(2, F, 4)
    feats64 = np.stack([tt, en, p, ent], -1).astype(np.float64)   # [M, 4]
    phi = np.einsum("mj,cfj->mcf", feats64, W64)                  # [M, 2, F]
    phi = phi.transpose(0, 2, 1).reshape(M, 2 * F)                # [M, 64]

    # compensated fp32 deltas: partial fp32 sums track the fp64 table
    dphi = np.empty((M, 2 * F), np.float32)
    running = np.zeros(2 * F, np.float64)
    for m in range(M):
        d = (phi[m] - running).astype(np.float32)
        dphi[m] = d
        running += d.astype(np.float64)

    # pad M to a multiple of 8 partitions; L pad = +inf (never <= x)
    mp = max(16, ((M + 7) // 8) * 8)
    L_pad = np.full(mp, np.float32(np.inf), np.float32)
    L_pad[:M] = L
    dphi_pad = np.zeros((mp, 2 * F), np.float32)
    dphi_pad[:M] = dphi

    def pack_hilo(d32):
        """[mp, 2F] fp32 -> [mp, 4F] fp16: per f (hi_c0, hi_c1, lo_c0, lo_c1)."""
        hi16 = d32.astype(np.float16)
        lo16 = (d32.astype(np.float64) - hi16.astype(np.float64)) \
            .astype(np.float16)
        out = np.empty((mp, 4 * F), np.float16)
        for f in range(F):
            out[:, 4 * f:4 * f + 2] = hi16[:, 2 * f:2 * f + 2]
            out[:, 4 * f + 2:4 * f + 4] = lo16[:, 2 * f:2 * f + 2]
        return out

    whi = pack_hilo(dphi_pad)                        # is_ge chunks
    whs = pack_hilo(0.5 * dphi_pad)                  # Sign chunks (0.5-scaled)

    # consts [mp, 130] fp32: col0 = L, col1 = -L, cols 2:66 = whi (f16 pairs
    # viewed as f32 words), cols 66:130 = whs
    consts = np.zeros((mp, 130), np.float32)
    consts[:, 0] = L_pad
    consts[:, 1] = -L_pad
    consts[:, 2:66] = whi.view(np.float32)
    consts[:, 66:130] = whs.view(np.float32)

    bcast_r, cmp_r = _routes()
    # Sign-path constant per channel: K_c = sum over sign-chunk features f of
    # sum_m [(0.5 d)_hi + (0.5 d)_lo]  (from the actual device fp16 tables)
    K = np.zeros(2, np.float64)
    sign_f = [f for f in range(F) if cmp_r[f // FPC] == "act"]
    for f in sign_f:
        for c in range(2):
            K[c] += (whs[:, 4 * f + c].astype(np.float64).sum()
                     + whs[:, 4 * f + 2 + c].astype(np.float64).sum())

    # exact x == L hits on Sign-path features lose 0.5*dphi (sign(0) = 0)
    corrections = []                                 # (n, f, m) triples
    hit_rows, hit_fs = np.nonzero(np.isin(x, L[:M]))
    for n, f in zip(hit_rows, hit_fs):
        if cmp_r[f // FPC] != "act":
            continue
        m = int(np.nonzero(L[:M] == x[n, f])[0][0])
        corrections.append((int(n), int(f), m))

    return consts, whi, whs, mp, K, corrections, bcast_r, cmp_r


# ----------------------------------------------------------------------------
# Device kernel
# ----------------------------------------------------------------------------

def _build_device_program(mp, bcast_r, cmp_r):
    nc = bacc.Bacc("TRN2", target_bir_lowering=False, debug=False,
                   num_devices=NCORES)
    f32 = mybir.dt.float32
    f16 = mybir.dt.float16
    bf16 = mybir.dt.bfloat16
    is_ge = mybir.AluOpType.is_ge
    Sign = mybir.ActivationFunctionType.Sign

    xs_d = nc.dram_tensor("xs", [3, E + mp], bf16, kind="ExternalInput").ap()
    ct_d = nc.dram_tensor("ct", [mp, 130], f32, kind="ExternalInput").ap()
    need_xf = any(r == "gps" for r in bcast_r)
    if need_xf:
        xf_d = nc.dram_tensor("xf", [1, E], f32, kind="ExternalInput").ap()
    out_d = nc.dram_tensor("out", [128, 8], f32, kind="ExternalOutput").ap()

    with tile.TileContext(nc) as tc, ExitStack() as ctx:
        consts = ctx.enter_context(tc.tile_pool(name="consts", bufs=1))
        warmp = ctx.enter_context(tc.tile_pool(name="warm", bufs=1))
        gpb = ctx.enter_context(tc.tile_pool(name="gpb", bufs=2))
        u16p = ctx.enter_context(tc.tile_pool(name="u16", bufs=8))
        outp = ctx.enter_context(tc.tile_pool(name="outp", bufs=1))
        psum = ctx.enter_context(tc.tile_pool(name="psum", bufs=3,
                                              space="PSUM"))
        psacc = ctx.enter_context(tc.tile_pool(name="psacc", bufs=1,
                                               space="PSUM"))
        pswarm = ctx.enter_context(tc.tile_pool(name="pswarm", bufs=1,
                                                space="PSUM"))

        # ---- input DMAs ------------------------------------------------
        # xf first on the Pool engine's own SWDGE queue (it is also the
        # consumer); xs + ct share the SP HWDGE queue.
        if need_xf:
            xf = consts.tile([1, E], f32, tag="xf")
            nc.gpsimd.dma_start(xf[:, :], xf_d)      # SWDGE, Pool engine
        xs = consts.tile([3, E + mp], bf16, tag="xs")
        nc.sync.dma_start(xs[:, :], xs_d)            # SP queue, fastest decode
        ct = consts.tile([mp, 130], f32, tag="ct")
        nc.sync.dma_start(ct[:, :], ct_d)            # SP queue, second

        lpe = ct[:, 0:1]
        nlpe = ct[:, 1:2]
        whi = ct[:, 2:66].bitcast(f16)               # [mp, 128]
        whs = ct[:, 66:130].bitcast(f16)
        ones3 = xs[:, E:E + mp]                      # [3, mp] all-ones bf16

        # ---- warmup: act table load + PE pstate ramp during the DMA head.
        # Const APs are dependency-free (initialized behind the program's
        # startup barrier), so these run immediately.
        # transposed accumulators: u16 is the stationary operand, the
        # [mp, 4] weight slice is the moving operand, so each accumulate
        # matmul streams only 4 columns.  accL: x-rows 0:128, accH: rows
        # 128:256 (per core), channels (hi0, hi1, lo0, lo1).
        accL = psacc.tile([128, 4], f32, tag="accL")
        accH = pswarm.tile([128, 4], f32, tag="accH")
        acc_n = [accL, accH]

        cf0 = nc.const_aps.aps[(f32, 0.0)][0:1, 0:1]
        cb1 = nc.const_aps.aps[(bf16, 1.0)][0:1, 0:1]
        wo = warmp.tile([1, 1], f16, tag="wo")
        nc.scalar.activation(wo[:, :], cf0, Sign, bias=0.0, scale=1.0)
        # warm matmul into accL's bank; its group closes long before the
        # real accumulation group starts (which resets the region anyway)
        nc.tensor.matmul(accL[0:1, 0:1], cb1, cb1, start=True, stop=True,
                         skip_group_check=True)

        # ---- pipeline ---------------------------------------------------
        xb_ps = {}                                   # chunk -> PSUM tile
        xb_sb = {}                                   # chunk -> SBUF tile
        u16 = {}

        def emit_bcast(k):
            sl = slice(k * CHUNK, (k + 1) * CHUNK)
            if bcast_r[k] == "pe":
                t = psum.tile([mp, CHUNK], f32, tag="xb", name=f"xb{k}")
                # one matmul per 512-col half: PSUM-bank limit
                for h in range(CHUNK // 512):
                    hs = slice(k * CHUNK + h * 512, k * CHUNK + (h + 1) * 512)
                    nc.tensor.matmul(t[:, h * 512:(h + 1) * 512],
                                     ones3[:, :], xs[:, hs],
                                     start=True, stop=True)
                xb_ps[k] = t
            else:
                t = gpb.tile([mp, CHUNK], f32, tag="xg", name=f"xg{k}")
                nc.gpsimd.partition_broadcast(t[:, :], xf[:, sl])
                xb_sb[k] = t

        def emit_cmp(k):
            src = xb_ps[k] if k in xb_ps else xb_sb[k]
            u = u16p.tile([mp, CHUNK], f16, tag="u", name=f"u{k}")
            if cmp_r[k] == "dve":
                nc.vector.tensor_scalar(u[:, :], src[:, :], lpe, None, is_ge)
            elif cmp_r[k] == "pool":
                nc.gpsimd.tensor_scalar(u[:, :], src[:, :], lpe, None, is_ge)
            elif k == 0 and os.environ.get("SPLIT0", "") == "1":
                # split the pipeline-fill chunk so its first half (and the
                # first accumulates) unblock one half-compare earlier
                h = CHUNK // 2
                nc.scalar.activation(u[:, 0:h], src[:, 0:h], Sign,
                                     bias=nlpe, scale=1.0)
                nc.scalar.activation(u[:, h:], src[:, h:], Sign,
                                     bias=nlpe, scale=1.0)
            else:
                nc.scalar.activation(u[:, :], src[:, :], Sign,
                                     bias=nlpe, scale=1.0)
            u16[k] = u

        acc_started = [False, False]

        def emit_acc(k, last_chunk):
            tbl = whs if cmp_r[k] == "act" else whi
            for half in range(FPC):
                f = FPC * k + half
                # on the final f-block, stop accH's group first so its copy
                # can overlap accL's last matmul
                nhs = (1, 0) if (last_chunk and half == FPC - 1) else (0, 1)
                for nh in nhs:
                    lo = half * N_LOC + nh * 128
                    usl = u16[k][:, lo:lo + 128]     # stationary [mp, 128]
                    last = (last_chunk and half == FPC - 1)
                    nc.tensor.matmul(acc_n[nh][:, :], usl,
                                     tbl[:, 4 * f:4 * f + 4],
                                     start=not acc_started[nh], stop=last)
                    acc_started[nh] = True

        pe_chunks = [k for k in range(NCHUNK) if bcast_r[k] == "pe"]
        gps_chunks = [k for k in range(NCHUNK) if bcast_r[k] == "gps"]
        acc_seq = _acc_order()
        lookahead = 3

        # gpsimd broadcasts stream independently on the Pool engine
        for k in gps_chunks:
            emit_bcast(k)
        # PE: prefetch broadcasts up to the psum buffer count
        for k in pe_chunks[:lookahead]:
            emit_bcast(k)
        # compares whose broadcast is already emitted, in per-engine chunk
        # order (each engine consumes them in this program order)
        emitted_b = set(gps_chunks) | set(pe_chunks[:lookahead])
        acc_pos = {k: i for i, k in enumerate(acc_seq)}
        for k in sorted(emitted_b, key=lambda k: acc_pos[k]):
            emit_cmp(k)
        # accumulates in ACC_ORDER; late broadcasts (+their compares)
        # interleave into the stream as buffers free up
        nxt = lookahead
        for i, k in enumerate(acc_seq):
            emit_acc(k, last_chunk=(i == NCHUNK - 1))
            if nxt < len(pe_chunks):
                kb = pe_chunks[nxt]
                emit_bcast(kb)
                emit_cmp(kb)
                nxt += 1

        # ---- output -----------------------------------------------------
        outs = outp.tile([128, 8], f32)
        nc.vector.tensor_scalar(outs[:, 4:8], accH[:, :], 0.0, None,
                                mybir.AluOpType.add)
        nc.vector.tensor_scalar(outs[:, 0:4], accL[:, :], 0.0, None,
                                mybir.AluOpType.add)
        nc.sync.dma_start(out_d, outs[:, :])

    nc.compile()
    return nc


# ----------------------------------------------------------------------------
# Entry point
# ----------------------------------------------------------------------------

def kernel(x, initial_cond, threshold, epsilon, W, b):
    global LAST_RESULTS, LAST_NC
    x = np.ascontiguousarray(np.asarray(x, np.float32))
    W = np.asarray(W, np.float32)
    b = np.asarray(b, np.float32)
    ic = float(np.asarray(initial_cond).reshape(-1)[0])
    thr = float(np.asarray(threshold).reshape(-1)[0])
    eps = float(np.asarray(epsilon).reshape(-1)[0])

    consts, whi, whs, mp, K, corrections, bcast_r, cmp_r = _build_tables(
        x, ic, thr, eps, W, b)
    need_xf = any(r == "gps" for r in bcast_r)

    nc = _build_device_program(mp, bcast_r, cmp_r)
    LAST_NC = nc

    bf = ml_dtypes.bfloat16
    in_maps = []
    for d in range(NCORES):
        xd = x[d * N_LOC:(d + 1) * N_LOC, :]         # [256, 32]
        xrow = np.ascontiguousarray(xd.T).reshape(E)  # f-major
        hi, mid, lo = _split_bf16_3(xrow)
        xsplit = np.ones((3, E + mp), bf)
        xsplit[0, :E] = hi
        xsplit[1, :E] = mid
        xsplit[2, :E] = lo
        im = {"xs": xsplit, "ct": consts}
        if need_xf:
            im["xf"] = xrow.reshape(1, E)
        in_maps.append(im)

    res = run_bass_kernel_spmd(nc, in_maps, core_ids=list(range(NCORES)))
    LAST_RESULTS = res

    out = np.empty((N, 2), np.float64)
    for d in range(NCORES):
        o = res.results[d]["out"].astype(np.float64)   # [128, 8]
        # cols 0:4 = x-rows 0:128, cols 4:8 = rows 128:256; per 4-block:
        # (hi_c0, hi_c1, lo_c0, lo_c1)
        for nh in range(2):
            blk = o[:, 4 * nh:4 * nh + 4]
            rows = slice(d * N_LOC + nh * 128, d * N_LOC + nh * 128 + 128)
            out[rows, :] = blk[:, 0:2] + blk[:, 2:4]
    out += b.astype(np.float64).reshape(1, 2) + K.reshape(1, 2)
    for (n, f, m) in corrections:
        for c in range(2):
            out[n, c] += (float(whs[m, 4 * f + c])
                          + float(whs[m, 4 * f + 2 + c]))
    return out.astype(np.float32)
